# revision 15
# baseline (speedup 1.0000x reference)
"""ComplexOscillator Trainium2 kernel (8-core SPMD, full-I/O contract).

kernel(frequencies[16,64,96000] f32, initial_phase[16,64,1] f32)
  -> cos(cumsum(2*pi/48000 * masked_f, t) + phase0), f32.

Sharding: batch*osc rows (1024) split across 8 cores -> 128 rows/core = one
SBUF partition per row; the time axis (cumsum axis) stays whole per core.

Core idea: the whole per-element computation except the final sine runs as
ONE custom DVE op (1 elem/cycle/partition):

    g      = f * (1/24000)              # "half-turn" units: one period == 2
    inc    = g if g < 1 else 0          # anti-alias mask (g<1 <=> f<24000,
                                        #   exact for every fp32 f: 24000*C1
                                        #   rounds to exactly 1.0)
    y[k]   = init + inc[0] + ... + inc[k]   # DVE scan, same-stage feedback
    out[k] = y[k] - ((y[k] + M2) - M2)      # wrap to [-1,1]: M2=1.5*2^24 has
                                        #   ulp 2, so (y+M2)-M2 rounds y to the
                                        #   nearest even integer; subtraction
                                        #   is exact => zero wrap error.

Tracking y = phase/pi + 1/2 (i.e. 2*(turns + 1/4)) makes the masked-out
contribution exactly 0 mod 2 and folds the cos->sin quarter-turn into the
initial state, so the output is simply sin(pi * u), u in [-1,1] -- inside the
ACT Sin LUT's valid [-pi, pi] domain.  Chunk carries reuse out[:, -1] (the
wrapped phase), so fp32 accumulation noise stays ~1e-3 rad over 96000 steps.

Engine budget per core: DVE ~1.04 cyc/elem (scan op), ACT 1 cyc/elem (Sin),
DMA 48 MB in (f32) + 24 MB out (fp16) -- memory-bound at ~200 us.
"""

import numpy as np
import sys
import os
import json

if "/opt/trn_rl_repo" not in sys.path:
    sys.path.insert(0, "/opt/trn_rl_repo")

import concourse.bass as bass
import concourse.bacc as bacc
import concourse.mybir as mybir
from concourse.tile import TileContext
from concourse.bass_utils import run_bass_kernel_spmd

P = 128
B, N, T = 16, 64, 96000
NCORES = 8
ROWS = B * N  # 1024

C1SCALE = float(np.float32(1.0 / 24000.0))  # f -> half-turns; 24000*C1 == 1.0
MAGIC2 = float(1.5 * 2**24)  # ulp 2: (y+M2)-M2 = round-to-nearest-even-int
PI = float(np.pi)
INV_PI = float(np.float32(1.0 / np.pi))

LAST_EXEC_NS = None
LAST_RESULTS = None

_OSC_OP = None


def _get_osc_op():
    """Define + register the fused mask/scale/scan/wrap custom DVE op.

    out[k] = y[k] - ((y[k] + imm2) - imm2),
    y[k] = s0 + sum_{i<=k} (in0[i]*s1 if in0[i]*s1 < 1 else 0)

    s0 (C0) carries the per-partition initial phase [P,1]; s1 (C1) is the
    scale; imm2 (C2) the wrap magic.  Registered into dve_ops.OPS at runtime
    so dve_table_for_ops can emit the per-NEFF uop table.
    """
    global _OSC_OP
    if _OSC_OP is not None:
        return _OSC_OP
    import concourse.dve_ops as dve_ops_mod
    from concourse.dve_spec import (
        C0, C1, C2, Src0, Zero, One, AluOp, Scan, Spec, select,
        _has_src1, lower,
    )
    from concourse.dve_uop import DveOpSpec

    name = "OSC_MASKED_SCAN_WRAP_ANT"
    if name in dve_ops_mod._SUB_OPCODE_FOR_NAME:
        _OSC_OP = next(op for op in dve_ops_mod.OPS if op.name == name)
        return _OSC_OP

    g = Src0 * C1
    masked = select(g < One, g, Zero)
    y = Scan(AluOp.ADD, masked, init=C0)
    body = y - ((y + C2) - C2)

    def ref(in0, in1, s0, s1, imm2):
        gg = (in0.astype(np.float32) * np.float32(s1)).astype(np.float32)
        m = np.where(gg < np.float32(1.0), gg, np.float32(0.0)).astype(np.float32)
        out = np.empty_like(m)
        if isinstance(s0, np.ndarray):
            state = s0.astype(np.float32).reshape(-1)
        else:
            state = np.full((m.shape[0],), np.float32(s0), np.float32)
        M = np.float32(imm2)
        for k in range(m.shape[-1]):
            state = (state + m[..., k]).astype(np.float32)
            t = (state + M).astype(np.float32)
            w = (t - M).astype(np.float32)
            out[..., k] = (state - w).astype(np.float32)
        return out

    spec = Spec(body=body, reference=ref)
    shas = {}
    for ver in ("v3", "v4"):
        try:
            sp = DveOpSpec(name=name, opcode=0, uops=lower(spec, ver=ver),
                           rd1_en=_has_src1(spec))
            shas[ver] = sp.sha(ver)
        except Exception:
            pass
    op = dve_ops_mod.DveOp(name, spec, subdim=False, uops_sha=shas)
    dve_ops_mod.OPS.append(op)
    dve_ops_mod.CUSTOM_DVE_SPECS[name] = spec
    dve_ops_mod._SUB_OPCODE_FOR_NAME[name] = (
        max(dve_ops_mod._SUB_OPCODE_FOR_NAME.values()) + 1
    )
    _OSC_OP = op
    return op


def _build(T=T, TC=4000, out_dt="fp16", repeat=1, split_dma=False, ubufs=3):
    assert T % TC == 0
    nchunks = T // TC
    osc_op = _get_osc_op()
    nc = bacc.Bacc()
    freq = nc.declare_dram_parameter("freq", [P, T], mybir.dt.float32,
                                     isOutput=False)
    ph0 = nc.declare_dram_parameter("ph0", [P, 1], mybir.dt.float32,
                                    isOutput=False)
    odt = {"fp16": mybir.dt.float16, "bf16": mybir.dt.bfloat16,
           "f32": mybir.dt.float32}[out_dt]
    outd = nc.declare_dram_parameter("out", [P, T], odt, isOutput=True)

    with TileContext(nc) as tc:
        with (
            tc.tile_pool(name="const", bufs=1) as cpool,
            tc.tile_pool(name="fin", bufs=3) as fpool,
            tc.tile_pool(name="u", bufs=ubufs) as upool,
            tc.tile_pool(name="o", bufs=3) as opool,
        ):
            ph0_t = cpool.tile([P, 1], mybir.dt.float32)
            nc.sync.dma_start(out=ph0_t[:], in_=ph0[:])
            # y0 = ph0/pi + 1/2  (phase in half-turns, +quarter-turn for
            # the cos->sin shift)
            y0 = cpool.tile([P, 1], mybir.dt.float32)
            nc.vector.tensor_scalar(y0[:], ph0_t[:], INV_PI, 0.5,
                                    op0=mybir.AluOpType.mult,
                                    op1=mybir.AluOpType.add)

            prev_u = None
            for jj in range(nchunks * repeat):
                j = jj % nchunks
                sl = slice(j * TC, (j + 1) * TC)
                f = fpool.tile([P, TC], mybir.dt.float32)
                if split_dma:
                    # Both streams column-split across both HWDGE queues:
                    # each queue carries (in+out)/2 per-partition bytes.
                    h = TC // 2
                    nc.sync.dma_start(out=f[:, 0:h], in_=freq[:, sl][:, 0:h])
                    nc.scalar.dma_start(out=f[:, h:TC],
                                        in_=freq[:, sl][:, h:TC])
                    out_eng = None
                else:
                    nc.sync.dma_start(out=f[:], in_=freq[:, sl])
                    out_eng = nc.scalar

                u = upool.tile([P, TC], mybir.dt.float32)
                init = y0[:, 0:1] if j == 0 else prev_u[:, TC - 1: TC]
                nc.vector._custom_dve(
                    osc_op, out=u[:], in0=f[:],
                    s0=init, s1=C1SCALE, imm2=MAGIC2,
                )
                prev_u = u

                o = opool.tile([P, TC], odt)
                nc.scalar.activation(
                    o[:], u[:], mybir.ActivationFunctionType.Sin,
                    bias=0.0, scale=PI,
                )
                if out_eng is None:
                    h = TC // 2
                    nc.sync.dma_start(out=outd[:, sl][:, 0:h], in_=o[:, 0:h])
                    nc.scalar.dma_start(out=outd[:, sl][:, h:TC],
                                        in_=o[:, h:TC])
                else:
                    out_eng.dma_start(out=outd[:, sl], in_=o[:])
    nc.compile()
    return nc


def kernel(frequencies: np.ndarray, initial_phase: np.ndarray) -> np.ndarray:
    global LAST_EXEC_NS, LAST_RESULTS
    f = np.ascontiguousarray(frequencies, dtype=np.float32).reshape(ROWS, T)
    p = np.ascontiguousarray(initial_phase, dtype=np.float32).reshape(ROWS, 1)

    build_kw = json.loads(os.environ.get("OSC_KW", "{}"))
    nc = _build(**build_kw)
    rows_per_core = ROWS // NCORES  # 128
    in_maps = []
    for c in range(NCORES):
        r0 = c * rows_per_core
        in_maps.append({
            "freq": f[r0: r0 + rows_per_core],
            "ph0": p[r0: r0 + rows_per_core],
        })

    trace = os.environ.get("OSC_TRACE", "0") == "1"
    res = run_bass_kernel_spmd(
        nc, in_maps, list(range(NCORES)), trace=trace,
    )
    LAST_EXEC_NS = res.exec_time_ns
    LAST_RESULTS = res
    out = np.empty((ROWS, T), dtype=np.float32)
    for c in range(NCORES):
        out[c * rows_per_core: (c + 1) * rows_per_core] = np.asarray(
            res.results[c]["out"], dtype=np.float32)
    return out.reshape(B, N, T)
